# revision 1
# baseline (speedup 1.0000x reference)
"""Causal self-attention (B=4, T=2048, C=1024, H=16) on 8 TRN2 NeuronCores.

Sharding: tensor-parallel over heads. Core i owns heads (2i, 2i+1), i.e. 128
of the 1024 q/k/v channels:
  - projections: qT/kT = (x @ W[:, ci:ci+128]).T computed as W_sliceT-stationary
    matmuls against a host-pre-transposed xT, giving [128, 8192] activations
    that live in SBUF for the whole kernel.  1/sqrt(hs) is folded into Wq/bq.
  - attention per (batch, head) with the score matrix built transposed
    (S^T[tk, tq]) so the P @ v contraction needs no on-chip transpose of P;
    softmax is computed without the running-max (logits are O(4) here) and the
    denominator falls out of a ones-column appended to v.  Both heads' scores
    share one 2-bank PSUM tile so a single ACT exp covers them.
  - output projection partial = y_heads @ Wv[rows ci:ci+128, :]; the 8 K-split
    partials are summed on the host (the "all-reduce" of this TP scheme), plus
    the final bias.

Engine placement: PE matmuls (incl. denominator broadcast via K=1 matmuls),
ACT exp only, DVE copies/reciprocal/normalize, GpSimd causal masks, DMA moves
partition-shifted rows (engines cannot shift partitions).

kernel() accepts the full unsharded inputs and returns the full output.
"""

import numpy as np
import ml_dtypes

P = 128
B, T, C, H = 4, 2048, 1024, 16
HS = C // H          # 64
NCORES = 8
TT = B * T           # 8192 tokens total
KT = C // P          # 8 contraction tiles for the projections
TKB = T // P         # 16 key tiles per batch
CH = 512             # tq chunk width
NCH = T // CH        # 4 tq chunks per batch

_CACHE = {}


def _build_nc():
    """Build + compile the single-core SPMD Bass program (same on all cores)."""
    from contextlib import ExitStack

    import concourse.mybir as mybir
    import concourse.tile as tile
    from concourse import bacc

    dt = mybir.dt
    BF = dt.bfloat16
    F32 = dt.float32
    AF = mybir.ActivationFunctionType
    ALU = mybir.AluOpType

    nc = bacc.Bacc("TRN2", target_bir_lowering=False, debug=False)

    xT = nc.dram_tensor("xT", [C, TT], BF, kind="ExternalInput").ap()
    wq = nc.dram_tensor("wq", [C, P], BF, kind="ExternalInput").ap()
    wk = nc.dram_tensor("wk", [C, P], BF, kind="ExternalInput").ap()
    wv = nc.dram_tensor("wv", [C, P], BF, kind="ExternalInput").ap()
    wvo = nc.dram_tensor("wvo", [P, C], BF, kind="ExternalInput").ap()
    bq = nc.dram_tensor("bq", [P, 1], F32, kind="ExternalInput").ap()
    bk = nc.dram_tensor("bk", [P, 1], F32, kind="ExternalInput").ap()
    bv = nc.dram_tensor("bv", [P, 1], F32, kind="ExternalInput").ap()
    msk = nc.dram_tensor("msk", [P, P], BF, kind="ExternalInput").ap()
    idn = nc.dram_tensor("idn", [P, P], BF, kind="ExternalInput").ap()
    out = nc.dram_tensor("out", [TT, C], BF, kind="ExternalOutput").ap()

    xT3 = xT.rearrange("(ko p) t -> p ko t", p=P)
    wq3 = wq.rearrange("(ko p) m -> p ko m", p=P)
    wk3 = wk.rearrange("(ko p) m -> p ko m", p=P)
    wv3 = wv.rearrange("(ko p) m -> p ko m", p=P)
    out3 = out.rearrange("(r p) c -> p r c", p=P)

    with tile.TileContext(nc) as tc, ExitStack() as ctx:
        pers = ctx.enter_context(tc.tile_pool(name="pers", bufs=1))

        wq_sb = pers.tile([P, KT, P], BF, tag="wq")
        wk_sb = pers.tile([P, KT, P], BF, tag="wk")
        wv_sb = pers.tile([P, KT, P], BF, tag="wv")
        wvo_sb = pers.tile([P, C], BF, tag="wvo")
        bq_sb = pers.tile([P, 1], F32, tag="bq")
        bk_sb = pers.tile([P, 1], F32, tag="bk")
        bv_sb = pers.tile([P, 1], F32, tag="bv")
        msk_sb = pers.tile([P, P], BF, tag="msk")
        idn_sb = pers.tile([P, P], BF, tag="idn")
        for dst, srcap in ((wq_sb, wq3), (idn_sb, idn), (bq_sb, bq),
                           (bk_sb, bk), (bv_sb, bv), (wk_sb, wk3),
                           (wv_sb, wv3), (msk_sb, msk), (wvo_sb, wvo)):
            nc.gpsimd.dma_start(dst[:], srcap)

        # all-ones; rows {0,32,64,96} used as K=1 stationaries that broadcast
        # a denominator-reciprocal row across 64 output partitions.
        ones97 = pers.tile([97, 64], BF, tag="ones97")
        nc.vector.memset(ones97[:], 1.0)

        # Persistent activations: rows 0-63 = even head, 64-127 = odd head.
        qT_sb = pers.tile([P, TT], BF, tag="qT")
        kT_sb = pers.tile([P, TT], BF, tag="kT")
        vT_sb = pers.tile([P, TT], BF, tag="vT")
        # v re-laid out [token, dim] per 128-token tile, with a ones column
        # per head for the softmax denominator.
        va_sb = pers.tile([P, B * TKB, 130], BF, tag="va")
        nc.vector.memset(va_sb[:, :, 64], 1.0)
        nc.vector.memset(va_sb[:, :, 129], 1.0)

        work = ctx.enter_context(tc.tile_pool(name="work", bufs=3))
        ptp = ctx.enter_context(tc.tile_pool(name="ptp", bufs=3))
        # PSUM: "s" merged A|B score tiles 2x2-bank, "y" accumulators 2,
        # "aux" (projections / transpose / broadcast / out-proj) 2 = 8 banks.
        sps = ctx.enter_context(tc.tile_pool(name="sps", bufs=2, space="PSUM"))
        yps = ctx.enter_context(tc.tile_pool(name="yps", bufs=2, space="PSUM"))
        aux = ctx.enter_context(tc.tile_pool(name="aux", bufs=2, space="PSUM"))

        def emit_proj(b):
            # ---- projections for batch b (4 chunks of 512 tokens) ----
            for cc in range(NCH):
                chi = b * NCH + cc
                sl = slice(chi * CH, (chi + 1) * CH)
                xch = work.tile([P, KT, CH], BF, tag="xch")
                for k in range(KT):
                    nc.sync.dma_start(xch[:, k], xT3[:, k, sl])
                for which in range(3):  # q, k, v
                    w_sb, o_sb, b_sb = (
                        (wq_sb, qT_sb, bq_sb), (wk_sb, kT_sb, bk_sb),
                        (wv_sb, vT_sb, bv_sb))[which]
                    pp = aux.tile([P, CH], F32, tag="aux", name="pp")
                    for k in range(KT):
                        nc.tensor.matmul(pp[:], w_sb[:, k], xch[:, k],
                                         start=(k == 0), stop=(k == KT - 1))
                    nc.vector.tensor_scalar_add(o_sb[:, sl], pp[:], b_sb[:])
                # transpose the 4 fresh v tiles into va_sb
                for g in range(chi * 4, chi * 4 + 4):
                    tp = aux.tile([P, CH], BF, tag="aux", name="tp")
                    nc.tensor.transpose(tp[:, :P], vT_sb[:, g * P:(g + 1) * P],
                                        idn_sb[:])
                    nc.vector.tensor_copy(
                        va_sb[:, g].rearrange("p (a c) -> p a c", a=2)[:, :, 0:64],
                        tp[:, :P].rearrange("p (a c) -> p a c", a=2))

        def emit_tloops(b, eager_tail=False):
            # ---- attention for batch b ----
            yT = work.tile([P, T], BF, tag="yT", name=f"yT{b}")
            ds = [work.tile([97, CH], BF, tag="ds", name=f"ds{b}_{h}")
                  for h in range(2)]
            nc.vector.memset(ds[0][:], 1.0)
            nc.vector.memset(ds[1][:], 1.0)
            for j in range(NCH):
                jsl = slice(j * CH, (j + 1) * CH)
                py = [yps.tile([P, CH], F32, tag="y", name=f"py{_h}")
                      for _h in range(2)]
                nlast = 4 * j + 3
                for t in range(4 * j + 4):
                    g = b * TKB + t
                    o = max(0, P * t - CH * j)
                    n = CH - o
                    tq0 = b * T + j * CH + o
                    ps = sps.tile([P, 2 * CH], F32, tag="s", name="ps")
                    pt = ptp.tile([P, 2 * CH], BF, tag="pt")
                    for h in (0, 1):
                        hoff = h * 64
                        nc.tensor.matmul(
                            ps[:, h * CH + o:(h + 1) * CH],
                            kT_sb[hoff:hoff + 64, g * P:(g + 1) * P],
                            qT_sb[hoff:hoff + 64, tq0:tq0 + n],
                            start=True, stop=True)
                    # one exp for both heads (3D AP over the two halves)
                    nc.scalar.activation(
                        pt.rearrange("p (a c) -> p a c", a=2)[:, :, o:CH],
                        ps.rearrange("p (a c) -> p a c", a=2)[:, :, o:CH],
                        AF.Exp)
                    if t >= 4 * j:  # causal boundary: triangle mask (DVE)
                        pt3 = pt.rearrange("p (a c) -> p a c", a=2)
                        nc.vector.tensor_tensor(
                            pt3[:, :, o:o + P], pt3[:, :, o:o + P],
                            msk_sb[:, None, :].to_broadcast((P, 2, P)),
                            ALU.mult)
                    for h in (0, 1):
                        nc.tensor.matmul(
                            py[h][:65, o:CH],
                            va_sb[:, g, 65 * h:65 * h + 65],
                            pt[:, h * CH + o:(h + 1) * CH],
                            start=(t == 0), stop=(t == nlast))
                # move unnormalized y + denominator rows off PSUM; DVE lanes
                # cannot shift partitions, DMA places the rows.
                for h in (0, 1):
                    tb = work.tile([65, CH], BF, tag="tb")
                    if h == 0:
                        nc.vector.tensor_copy(tb[:], py[h][0:65, :])
                    else:
                        nc.scalar.copy(tb[:], py[h][0:65, :])
                    nc.sync.dma_start(yT[h * 64:(h + 1) * 64, jsl], tb[0:64, :])
                    # stack denominator rows at partitions {0,32,64,96}
                    nc.sync.dma_start(ds[h][32 * j:32 * j + 1, :], tb[64:65, :])
                if eager_tail:
                    # last batch: normalize + project this chunk right away so
                    # the kernel's final stretch isn't one long serial tail.
                    rr = make_rr(b, ds, j)
                    norm_chunk(b, yT, rr, j)
                    proj_chunk(b, yT, j)

            return yT, ds

        def norm_chunk(b, yT, rr, j):
            jsl = slice(j * CH, (j + 1) * CH)
            for h in (0, 1):
                # broadcast lands on partitions h*64..h*64+64 so the
                # in-place multiply keeps matching partition bases.
                rp = aux.tile([P, CH], F32, tag="aux", name="rp")
                nc.tensor.matmul(rp[h * 64:(h + 1) * 64, :],
                                 ones97[32 * j:32 * j + 1, :],
                                 rr[h][32 * j:32 * j + 1, :],
                                 start=True, stop=True,
                                 tile_position=(32 * j, h * 64))
                nc.vector.tensor_mul(yT[h * 64:(h + 1) * 64, jsl],
                                     yT[h * 64:(h + 1) * 64, jsl],
                                     rp[h * 64:(h + 1) * 64, :])

        def make_rr(b, ds, j):
            rr = [work.tile([97, CH], BF, tag="rr", name=f"rr{b}_{j}_{h}")
                  for h in range(2)]
            with nc.allow_low_precision(reason="softmax denom"):
                nc.vector.reciprocal(rr[0][:], ds[0][:])
                nc.vector.reciprocal(rr[1][:], ds[1][:])
            return rr

        def emit_tail(b, yT, ds):
            rr = make_rr(b, ds, 0)
            for j in range(NCH):
                norm_chunk(b, yT, rr, j)
            for j in range(NCH):
                proj_chunk(b, yT, j)

        def proj_chunk(b, yT, j):
            # ---- output projection for one 512-token chunk of batch b ----
            if True:
                for half_i in range(2):
                    ost = work.tile([P, 4, CH], BF, tag="ost")
                    for g4 in range(4):
                        tt0 = j * CH + g4 * P
                        po = aux.tile([P, CH], F32, tag="aux", name="po")
                        nc.tensor.matmul(
                            po[:, :], yT[:, tt0:tt0 + P],
                            wvo_sb[:, half_i * CH:(half_i + 1) * CH],
                            start=True, stop=True)
                        if half_i == 0:
                            nc.vector.tensor_copy(ost[:, g4], po[:, :])
                        else:
                            nc.scalar.copy(ost[:, g4], po[:, :])
                    r0 = b * TKB + j * 4
                    nc.sync.dma_start(
                        out3[:, r0:r0 + 4, half_i * CH:(half_i + 1) * CH],
                        ost[:])

        state = {}
        for b in range(B):
            emit_proj(b)
            state[b] = emit_tloops(b, eager_tail=False)
            if b >= 1 and (b - 1) in state:
                emit_tail(b - 1, *state.pop(b - 1))
        if (B - 1) in state:
            emit_tail(B - 1, *state.pop(B - 1))

    nc.compile()
    return nc


def get_nc():
    if "nc" not in _CACHE:
        _CACHE["nc"] = _build_nc()
    return _CACHE["nc"]


def make_in_maps(inputs):
    bf16 = ml_dtypes.bfloat16
    f32 = np.float32
    x = np.asarray(inputs["x"], f32)
    Wq = np.asarray(inputs["Wq"], f32)
    Wk = np.asarray(inputs["Wk"], f32)
    Wv = np.asarray(inputs["Wv"], f32)
    bq = np.asarray(inputs["bq"], f32)
    bk = np.asarray(inputs["bk"], f32)
    bv = np.asarray(inputs["bv"], f32)

    scale = 1.0 / np.sqrt(HS)
    xT = np.ascontiguousarray(x.reshape(TT, C).T).astype(bf16)
    msk = np.triu(np.ones((P, P), f32)).astype(bf16)   # [p, f] = 1 iff f >= p
    idn = np.eye(P, dtype=f32).astype(bf16)

    in_maps = []
    for i in range(NCORES):
        cs = slice(i * P, (i + 1) * P)
        in_maps.append({
            "xT": xT,
            "wq": np.ascontiguousarray(Wq[:, cs] * scale).astype(bf16),
            "wk": np.ascontiguousarray(Wk[:, cs]).astype(bf16),
            "wv": np.ascontiguousarray(Wv[:, cs]).astype(bf16),
            "wvo": np.ascontiguousarray(Wv[cs, :]).astype(bf16),
            "bq": np.ascontiguousarray((bq[cs] * scale).reshape(P, 1)),
            "bk": np.ascontiguousarray(bk[cs].reshape(P, 1)),
            "bv": np.ascontiguousarray(bv[cs].reshape(P, 1)),
            "msk": msk,
            "idn": idn,
        })
    return in_maps


def run(inputs, **spmd_kwargs):
    """Run on the 8 cores; returns (full_output, BassKernelResults)."""
    from concourse.bass_utils import run_bass_kernel_spmd

    nc = get_nc()
    in_maps = make_in_maps(inputs)
    res = run_bass_kernel_spmd(nc, in_maps, core_ids=list(range(NCORES)),
                               **spmd_kwargs)
    acc = res.results[0]["out"].astype(np.float32).copy()
    for r in res.results[1:]:
        acc += r["out"]
    acc += np.asarray(inputs["bv"], np.float32)[None, :]
    return acc.reshape(B, T, C), res


def kernel(**inputs) -> np.ndarray:
    out, _ = run(inputs)
    return out

